# revision 58
# baseline (speedup 1.0000x reference)
"""DirGCNConv on 8 Trainium2 NeuronCores.

Math (reference):
  A = dense 0/1 adjacency from edge_index (coalesced), At = A.T
  SO_in  = mask(At@A),  SO_out = mask(A@At)   (mask: zero where edge / diagonal)
  y = 0.35*h1 + 0.35*h2 + 0.15*h3 + 0.15*h4,  h = dir_norm(M) @ x @ W.T + b

Sharding: each core c owns output rows Rc = [512c, 512c+512).
Everything on device is kept in a transposed "C layout" [K on partitions,
row-block m on free] so matmul lhsT operands never need transposing:
  C_in  = (At@A)[:, Rc] = SO_in[Rc, :].T  (pre-mask symmetry)
  C_out = (A@At)[:, Rc] = SO_out[Rc, :].T

All heavy matmuls run fp8 e4m3 DoubleRow (2 K-chunks per instr):
  - phase 1 (dense second-order blocks): 0/1 adjacency exact in fp8.
  - FO SpMMs: lhsT = host-split hi+lo fp8 of x*scale*8, rhs = resident
    0/1 fp8 column blocks.
  - SO SpMMs: lhsT = single fp8 of x*iso*32 (vector-split on device);
    rhs = exact integer-count fp8 mc.
  Range scales (8/32) keep fp8 operands out of the denormal zone and are
  folded into the per-node output scales.
Edge+diag masks are one fused vector op (host bakes the diagonal into the
fp8 mask stream). Row sums are fp8-DR ones-matmuls over slab pairs. Output
tails swap matmul roles (lhsT=aggT chunk, rhs=W.T chunk) so h comes out
node-major: no PE transposes, per-node scales are per-partition scalars.

Phase order hides both column-sum AllReduces under PE work:
  FO_s2d | P1(out) -> AR(out) | P1(in) -> AR(in) | FO_d2s | SO_out | SO_in.
"""
import numpy as np
import ml_dtypes
from contextlib import ExitStack

N = 4096
P = 128
KC = N // P          # 32 k-chunks
JC = KC // 2         # 16 DoubleRow pair-chunks
B = 512              # rows per core
MC = B // P          # 4 row chunks per core
D = 256
DH = D // P          # 2 feature chunks
NCORES = 8
FO_S = 8.0           # fp8 range scale for first-order x
SO_S = 32.0          # fp8 range scale for second-order x

_CACHE = {}


def _build_nc():
    import concourse.bacc as bacc
    import concourse.mybir as mybir
    import concourse.tile as tile
    from concourse.alu_op_type import AluOpType as op
    import bass_rust
    AF = bass_rust.ActivationFunctionType
    AX = bass_rust.AxisListType
    f32 = mybir.dt.float32
    bf16 = mybir.dt.bfloat16
    f8 = mybir.dt.float8e4
    DR = mybir.MatmulPerfMode.DoubleRow

    nc = bacc.Bacc("TRN2", num_devices=NCORES)

    a_strips = nc.dram_tensor("a_strips", [KC, P, KC, P], f8, kind="ExternalInput")
    at_strips = nc.dram_tensor("at_strips", [KC, P, KC, P], f8, kind="ExternalInput")
    acol8_d = nc.dram_tensor("acol8", [N, B], f8, kind="ExternalInput")
    atcol8_d = nc.dram_tensor("atcol8", [N, B], f8, kind="ExternalInput")
    amask8_d = nc.dram_tensor("amask8", [N, B], f8, kind="ExternalInput")
    atmask8_d = nc.dram_tensor("atmask8", [N, B], f8, kind="ExternalInput")
    xbf_d = nc.dram_tensor("xbf", [N, D], bf16, kind="ExternalInput")
    xf = {}
    for t in ("s2d", "d2s"):
        for h in ("hi", "lo"):
            # pair-packed [j, p, r, d]: 512B contiguous per partition per DMA
            xf[(t, h)] = nc.dram_tensor(f"x_{t}_{h}", [JC, P, 2, D], f8,
                                        kind="ExternalInput")
    oa_s2d_d = nc.dram_tensor("oa_s2d", [P, MC], f32, kind="ExternalInput")
    oa_d2s_d = nc.dram_tensor("oa_d2s", [P, MC], f32, kind="ExternalInput")
    wsrcT_d = nc.dram_tensor("wsrcT", [D, D], bf16, kind="ExternalInput")
    wdstT_d = nc.dram_tensor("wdstT", [D, D], bf16, kind="ExternalInput")
    y_d = nc.dram_tensor("y", [B, D], f32, kind="ExternalOutput")

    with tile.TileContext(nc) as tc:
        with ExitStack() as ctx:
            cpool = ctx.enter_context(tc.tile_pool(name="const", bufs=1))
            strips = ctx.enter_context(tc.tile_pool(name="strips", bufs=11))
            xw = ctx.enter_context(tc.tile_pool(name="xw", bufs=8))
            tiny = ctx.enter_context(tc.tile_pool(name="tiny", bufs=1))
            ps_fo = ctx.enter_context(tc.tile_pool(name="ps_fo", bufs=2, space="PSUM"))
            ps_c = ctx.enter_context(tc.tile_pool(name="ps_c", bufs=2, space="PSUM"))
            ps_rs = ctx.enter_context(tc.tile_pool(name="ps_rs", bufs=2, space="PSUM"))
            ps_y = ctx.enter_context(tc.tile_pool(name="ps_y", bufs=2, space="PSUM"))
            dram = ctx.enter_context(tc.tile_pool(name="dram", bufs=1, space="DRAM"))

            # ---- resident constants ----
            acol8_sb = cpool.tile([P, KC, B], f8, name="acol8_sb")
            atcol8_sb = cpool.tile([P, KC, B], f8, name="atcol8_sb")
            for j in range(JC):
                nc.gpsimd.dma_start(
                    out=atcol8_sb[:, 2 * j:2 * j + 2, :],
                    in_=atcol8_d.rearrange("(kc p) j -> p kc j", p=P)[:, 2 * j:2 * j + 2, :])
            for j in range(JC):
                nc.gpsimd.dma_start(
                    out=acol8_sb[:, 2 * j:2 * j + 2, :],
                    in_=acol8_d.rearrange("(kc p) j -> p kc j", p=P)[:, 2 * j:2 * j + 2, :])
            oa_sb = {}
            for name, dd in (("s2d", oa_s2d_d), ("d2s", oa_d2s_d)):
                t = cpool.tile([P, MC], f32, tag=f"oa_{name}", name=f"oa_{name}")
                nc.scalar.dma_start(out=t[:], in_=dd[:])
                oa_sb[name] = t
            w_sb = {}
            for name, dd in (("src", wsrcT_d), ("dst", wdstT_d)):
                t = cpool.tile([P, DH, D], bf16, tag=f"w_{name}", name=f"w_{name}")
                nc.scalar.dma_start(out=t[:], in_=dd.rearrange("(kc p) j -> p kc j", p=P))
                w_sb[name] = t
            ones2 = cpool.tile([P, 2, 16], f8, name="ones2")
            nc.vector.memset(ones2[:], 1.0)

            mc_sb = {"in": cpool.tile([P, KC, B], f8, tag="mcin", name="mcin"),
                     "out": cpool.tile([P, KC, B], f8, tag="mcout", name="mcout")}
            xfs = {(t, h): cpool.tile([P, JC, 2, D], f8, tag=f"xfs_{t}_{h}",
                                      name=f"xfs_{t}_{h}")
                   for t in ("s2d", "d2s") for h in ("hi", "lo")}
            xbf_sb = cpool.tile([P, KC, D], bf16, name="xbf_sb")
            nc.gpsimd.dma_start(out=xbf_sb[:],
                                in_=xbf_d.rearrange("(kc p) d -> p kc d", p=P))
            TERMS = ("fo_s2d", "fo_d2s", "so_in", "so_out")
            aggT = {t: cpool.tile([P, DH, B], bf16, tag=f"agg_{t}", name=f"agg_{t}")
                    for t in TERMS}
            colp = {s: cpool.tile([P, KC], f32, tag=f"colp_{s}", name=f"colp_{s}")
                    for s in ("in", "out")}
            iso_sb = {s: cpool.tile([P, KC], f32, tag=f"iso_{s}", name=f"iso_{s}")
                      for s in ("in", "out")}
            oso_sb = {s: cpool.tile([P, MC], f32, tag=f"oso_{s}", name=f"oso_{s}")
                      for s in ("in", "out")}
            ysb = cpool.tile([P, MC, D], f32)

            cc = {s: {"i": dram.tile([N], f32, tag=f"cc_i_{s}", name=f"cc_i_{s}"),
                      "o": dram.tile([N], f32, tag=f"cc_o_{s}", name=f"cc_o_{s}")}
                  for s in ("in", "out")}
            oso_dram = dram.tile([2, B], f32)

            from concourse.tile_rust import add_dep_helper
            ev_trace = {}

            # ============ phase 1: C blocks + mask + degree sums ============
            def phase1(side, strips_d, col8sb, mask_d):
                """C = (Mt@M)[:, Rc] via fp8 DoubleRow; mask+evict fused (host
                bakes the diagonal into mask_d); rowsums as fp8-DR ones-matmul."""
                mc = mc_sb[side]
                rs = ps_rs.tile([1, B], f32, tag="rs", name=f"rs_{side}")
                for i in range(KC):
                    strip = strips.tile([P, KC, P], f8, tag="strip", name="strip")
                    # alternate queues: one queue can't sustain the strip rate
                    (nc.sync if i % 2 == 0 else nc.scalar).dma_start(
                        out=strip[:], in_=strips_d[i])
                    mchk = strips.tile([P, B], f8, tag="mchk", name="mchk", bufs=8)
                    nc.scalar.dma_start(out=mchk[:], in_=mask_d[i * P:(i + 1) * P, :])
                    cps = ps_c.tile([P, B], f32, tag="c", name="cps")
                    for j in range(JC):
                        nc.tensor.matmul(cps[:], lhsT=strip[:, 2 * j:2 * j + 2, :],
                                         rhs=col8sb[:, 2 * j:2 * j + 2, :],
                                         perf_mode=DR,
                                         start=(j == 0), stop=(j == JC - 1))
                    # fused evict: zero where edge or diagonal (mask != 0)
                    mk = nc.vector.scalar_tensor_tensor(out=mc[:, i, :], in0=mchk[:],
                                                        scalar=0.0, in1=cps[:],
                                                        op0=op.is_equal, op1=op.mult)
                    ev_trace[(side, i)] = mk
                    # partial column sums (free-dim reduce)
                    nc.vector.reduce_sum(colp[side][:, i:i + 1], mc[:, i, :], axis=AX.X)
                    # row-sum ones-matmul over slab pairs, fp8 DoubleRow
                    if i % 2 == 1:
                        nc.tensor.matmul(rs[:], lhsT=ones2[:, :, :1],
                                         rhs=mc[:, i - 1:i + 1, :], perf_mode=DR,
                                         start=(i == 1), stop=(i == KC - 1))
                # o_so = (0.15/SO_S) * rsqrt(rowsum) * (rowsum > 0)
                ind = tiny.tile([1, B], f32, tag=f"rind_{side}", name=f"rind_{side}")
                nc.vector.tensor_scalar(out=ind[:], in0=rs[:], scalar1=0.0,
                                        scalar2=None, op0=op.is_gt)
                val = tiny.tile([1, B], f32, tag=f"rval_{side}", name=f"rval_{side}")
                nc.vector.tensor_scalar(out=val[:], in0=rs[:], scalar1=1e-30,
                                        scalar2=None, op0=op.max)
                nc.scalar.activation(out=val[:], in_=val[:],
                                     func=AF.Abs_reciprocal_sqrt,
                                     scale=(SO_S / 0.15) ** 2)
                nc.vector.tensor_tensor(out=val[:], in0=val[:], in1=ind[:], op=op.mult)
                si = 0 if side == "in" else 1
                nc.gpsimd.dma_start(out=oso_dram[si], in_=val[:])
                nc.gpsimd.dma_start(out=oso_sb[side][:],
                                    in_=oso_dram[si].rearrange("(mc p) -> p mc", p=P))
                # ship partial colsums + AllReduce (overlapped with later PE work)
                nc.gpsimd.dma_start(out=cc[side]["i"].rearrange("(kc p) -> p kc", p=P),
                                    in_=colp[side][:])
                nc.gpsimd.collective_compute(
                    "AllReduce", mybir.AluOpType.add,
                    replica_groups=[list(range(NCORES))],
                    ins=[cc[side]["i"].opt()], outs=[cc[side]["o"].opt()])

            def iso_prep(side, gate):
                """iso = SO_S * rsqrt(colsum) gated; `gate` keeps the readback
                from stalling the vector FIFO on collective latency."""
                raw = tiny.tile([P, KC], f32, tag=f"israw_{side}", name=f"israw_{side}")
                dma = nc.gpsimd.dma_start(out=raw[:],
                                          in_=cc[side]["o"].rearrange("(kc p) -> p kc", p=P))
                if gate is not None:
                    add_dep_helper(dma.ins, gate.ins, reason="iso readback after vector work")
                ind = tiny.tile([P, KC], f32, tag=f"isind_{side}", name=f"isind_{side}")
                nc.vector.tensor_scalar(out=ind[:], in0=raw[:], scalar1=0.0,
                                        scalar2=None, op0=op.is_gt)
                nc.vector.tensor_scalar(out=raw[:], in0=raw[:], scalar1=1e-30,
                                        scalar2=None, op0=op.max)
                nc.scalar.activation(out=raw[:], in_=raw[:],
                                     func=AF.Abs_reciprocal_sqrt,
                                     scale=1.0 / (SO_S * SO_S))
                nc.vector.tensor_tensor(out=iso_sb[side][:], in0=raw[:], in1=ind[:],
                                        op=op.mult)

            # SO SpMM: single-fp8 x*iso lhsT against resident fp8 mc, DoubleRow.
            # The two split ops of each pair go to different engines.
            def spmm_so(side, ps):
                scale, rhs_sb = iso_sb[side], mc_sb[side]
                last = None
                for j in range(JC):
                    x8 = xw.tile([P, 2, D], f8, tag="x8", name="x8")
                    for r in range(2):
                        k = 2 * j + r
                        if r == 0:
                            last = nc.vector.tensor_scalar(out=x8[:, r, :],
                                                           in0=xbf_sb[:, k, :],
                                                           scalar1=scale[:, k:k + 1],
                                                           scalar2=None, op0=op.mult)
                        else:
                            nc.scalar.activation(out=x8[:, r, :],
                                                 in_=xbf_sb[:, k, :],
                                                 func=AF.Copy,
                                                 scale=scale[:, k:k + 1])
                    for dh in range(DH):
                        nc.tensor.matmul(ps[dh][:],
                                         lhsT=x8[:, :, dh * P:(dh + 1) * P],
                                         rhs=rhs_sb[:, 2 * j:2 * j + 2, :],
                                         perf_mode=DR,
                                         start=(j == 0), stop=(j == JC - 1))
                return last

            # FO SpMM: host-split hi/lo fp8 lhsT, prefetched resident in SBUF
            # during phase 1 so the matmuls are never DMA-fed
            def spmm_fo(term, rhs_sb, ps):
                for j in range(JC):
                    for dh in range(DH):
                        for half, h in enumerate(("hi", "lo")):
                            nc.tensor.matmul(ps[dh][:],
                                             lhsT=xfs[(term, h)][:, j, :, dh * P:(dh + 1) * P],
                                             rhs=rhs_sb[:, 2 * j:2 * j + 2, :],
                                             perf_mode=DR,
                                             start=(j == 0 and half == 0),
                                             stop=(j == JC - 1 and half == 1))

            # output tail: h[Rc] = aggT.T @ W.T, node-major, no transposes
            TW = {"fo_s2d": "src", "fo_d2s": "dst", "so_out": "src", "so_in": "dst"}

            def term_tail(term, first, emit_y=False):
                w = w_sb[TW[term]]
                ot = {"fo_s2d": oa_sb["s2d"], "fo_d2s": oa_sb["d2s"],
                      "so_out": oso_sb["out"], "so_in": oso_sb["in"]}[term]
                for mh in range(MC):
                    g = ps_y.tile([P, D], f32, tag="y", name="gy")
                    for kh in range(DH):
                        nc.tensor.matmul(g[:], lhsT=aggT[term][:, kh, mh * P:(mh + 1) * P],
                                         rhs=w[:, kh, :],
                                         start=(kh == 0), stop=(kh == DH - 1))
                    dst = ysb[:, mh, :]
                    if first:
                        nc.vector.tensor_scalar(out=dst, in0=g[:],
                                                scalar1=ot[:, mh:mh + 1],
                                                scalar2=None, op0=op.mult)
                    else:
                        nc.vector.scalar_tensor_tensor(out=dst, in0=g[:],
                                                       scalar=ot[:, mh:mh + 1],
                                                       in1=dst, op0=op.mult,
                                                       op1=op.add)
                    if emit_y:
                        nc.gpsimd.dma_start(
                            out=y_d.rearrange("(mc p) d -> p mc d", p=P)[:, mh, :],
                            in_=ysb[:, mh, :])

            # ================= emission order =================
            # Phase 1 first: strips own the early HBM bandwidth; the FO SpMMs
            # run at the end where DMA queues are idle and cover AR(in).
            phase1("out", at_strips, atcol8_sb, atmask8_d)  # ends with AR(out) kickoff
            # FO x-streams prefetch on the idle gpsimd queue, flowing under P1(in)
            for t in ("s2d", "d2s"):
                for h in ("hi", "lo"):
                    for j in range(JC):
                        nc.gpsimd.dma_start(out=xfs[(t, h)][:, j, :, :],
                                            in_=xf[(t, h)][j])
            phase1("in", a_strips, acol8_sb, amask8_d)      # covers AR(out); kicks AR(in)

            iso_prep("out", gate=ev_trace[("in", KC - 8)])
            soout_ps = [ps_fo.tile([P, B], f32, tag="fo", name=f"soout_{dh}")
                        for dh in range(DH)]
            ev2 = spmm_so("out", soout_ps)
            for dh in range(DH):
                nc.vector.tensor_copy(out=aggT["so_out"][:, dh, :], in_=soout_ps[dh][:])
            term_tail("so_out", first=True)

            # iso(in) early: its vector chain runs under the FO SpMMs
            iso_prep("in", gate=ev2)

            fo_ps = {(t, dh): ps_fo.tile([P, B], f32, tag="fo", name=f"fo_{t}_{dh}")
                     for t in ("s2d", "d2s") for dh in range(DH)}
            spmm_fo("s2d", atcol8_sb, [fo_ps[("s2d", dh)] for dh in range(DH)])
            for dh in range(DH):
                nc.vector.tensor_copy(out=aggT["fo_s2d"][:, dh, :],
                                      in_=fo_ps[("s2d", dh)][:])
            term_tail("fo_s2d", first=False)

            spmm_fo("d2s", acol8_sb, [fo_ps[("d2s", dh)] for dh in range(DH)])
            for dh in range(DH):
                nc.vector.tensor_copy(out=aggT["fo_d2s"][:, dh, :],
                                      in_=fo_ps[("d2s", dh)][:])
            term_tail("fo_d2s", first=False)

            soin_ps = [ps_fo.tile([P, B], f32, tag="fo", name=f"soin_{dh}")
                       for dh in range(DH)]
            spmm_so("in", soin_ps)
            for dh in range(DH):
                nc.vector.tensor_copy(out=aggT["so_in"][:, dh, :], in_=soin_ps[dh][:])
            term_tail("so_in", first=False, emit_y=True)

    nc.finalize()
    return nc


def _host_prep(x, edge_index):
    ei = np.asarray(edge_index).astype(np.int64)
    lin = ei[0] * N + ei[1]
    uniq = np.unique(lin)
    A = np.zeros(N * N, np.float32)
    A[uniq] = 1.0
    A = A.reshape(N, N)
    dr = np.bincount((uniq // N).astype(np.int64), minlength=N).astype(np.float64)
    dc = np.bincount((uniq % N).astype(np.int64), minlength=N).astype(np.float64)

    def rnorm(d):
        return np.where(d > 0, 1.0 / np.sqrt(np.maximum(d, 1e-30)), 0.0).astype(np.float32)

    rdr, rdc = rnorm(dr), rnorm(dc)
    f8 = ml_dtypes.float8_e4m3
    A8 = A.astype(f8)
    At8 = np.ascontiguousarray(A8.T)
    # masks with the diagonal baked in (nonzero => zero the C entry)
    Am = A.copy()
    np.fill_diagonal(Am, 1.0)
    Am8 = Am.astype(f8)
    Atm8 = np.ascontiguousarray(Am8.T)
    a_strips = np.ascontiguousarray(A8.reshape(KC, P, KC, P).transpose(2, 1, 0, 3))
    at_strips = np.ascontiguousarray(At8.reshape(KC, P, KC, P).transpose(2, 1, 0, 3))
    mats = {"A8": A8, "At8": At8, "Am8": Am8, "Atm8": Atm8}
    return mats, a_strips, at_strips, rdr, rdc


def _fo_split(x, scale):
    f8 = ml_dtypes.float8_e4m3
    xs = (x * (FO_S * scale)[:, None]).astype(np.float32)
    hi = xs.astype(f8)
    lo = (xs - hi.astype(np.float32)).astype(f8)

    def pack(a):
        # [N, D] -> [JC, P, 2, D]: DMA-friendly pair-packed layout
        return np.ascontiguousarray(
            a.reshape(JC, 2, P, D).transpose(0, 2, 1, 3))

    return pack(hi), pack(lo)


def _in_maps(x, mats, a_strips, at_strips, rdr, rdc, wsrcT, wdstT):
    bf16 = ml_dtypes.bfloat16
    xs2d_hi, xs2d_lo = _fo_split(x, rdc)
    xd2s_hi, xd2s_lo = _fo_split(x, rdr)
    xbf = x.astype(bf16)
    w_src = np.ascontiguousarray(wsrcT).astype(bf16)
    w_dst = np.ascontiguousarray(wdstT).astype(bf16)
    maps = []
    for c in range(NCORES):
        sl = slice(c * B, (c + 1) * B)
        maps.append({
            "a_strips": a_strips, "at_strips": at_strips,
            "acol8": np.ascontiguousarray(mats["A8"][:, sl]),
            "atcol8": np.ascontiguousarray(mats["At8"][:, sl]),
            "amask8": np.ascontiguousarray(mats["Am8"][:, sl]),
            "atmask8": np.ascontiguousarray(mats["Atm8"][:, sl]),
            "xbf": xbf,
            "x_s2d_hi": xs2d_hi, "x_s2d_lo": xs2d_lo,
            "x_d2s_hi": xd2s_hi, "x_d2s_lo": xd2s_lo,
            "oa_s2d": np.ascontiguousarray((0.35 / FO_S * rdr[sl]).reshape(MC, P).T),
            "oa_d2s": np.ascontiguousarray((0.35 / FO_S * rdc[sl]).reshape(MC, P).T),
            "wsrcT": w_src, "wdstT": w_dst,
        })
    return maps


def kernel(x, edge_index, W_src, b_src, W_dst, b_dst):
    from concourse.bass_utils import run_bass_kernel_spmd

    x = np.asarray(x, dtype=np.float32)
    W_src = np.asarray(W_src, dtype=np.float32)
    W_dst = np.asarray(W_dst, dtype=np.float32)
    b_src = np.asarray(b_src, dtype=np.float32)
    b_dst = np.asarray(b_dst, dtype=np.float32)

    mats, a_strips, at_strips, rdr, rdc = _host_prep(x, edge_index)
    in_maps = _in_maps(x, mats, a_strips, at_strips, rdr, rdc,
                       np.ascontiguousarray(W_src.T), np.ascontiguousarray(W_dst.T))

    if "nc" not in _CACHE:
        _CACHE["nc"] = _build_nc()
    res = run_bass_kernel_spmd(_CACHE["nc"], in_maps, list(range(NCORES)))
    y = np.concatenate([res.results[c]["y"] for c in range(NCORES)], axis=0)
    y = y + 0.5 * (b_src + b_dst)[None, :]
    return np.ascontiguousarray(y.astype(np.float32))


# revision 61
# speedup vs baseline: 1.1923x; 1.1923x over previous
"""DirGCNConv on 8 Trainium2 NeuronCores.

Math (reference):
  A = dense 0/1 adjacency from edge_index (coalesced), At = A.T
  SO_in  = mask(At@A),  SO_out = mask(A@At)   (mask: zero where edge / diagonal)
  y = 0.35*h1 + 0.35*h2 + 0.15*h3 + 0.15*h4,  h = dir_norm(M) @ x @ W.T + b

Sharding: each core c owns output rows Rc = [512c, 512c+512).
Everything on device is kept in a transposed "C layout" [K on partitions,
row-block m on free] so matmul lhsT operands never need transposing:
  C_in  = (At@A)[:, Rc] = SO_in[Rc, :].T  (pre-mask symmetry)
  C_out = (A@At)[:, Rc] = SO_out[Rc, :].T

All heavy matmuls run fp8 e4m3 DoubleRow (2 K-chunks per instr):
  - phase 1 (dense second-order blocks): 0/1 adjacency exact in fp8.
  - FO SpMMs: lhsT = host-split hi+lo fp8 of x*scale*8, rhs = resident
    0/1 fp8 column blocks.
  - SO SpMMs: lhsT = single fp8 of x*iso*32 (vector-split on device);
    rhs = exact integer-count fp8 mc.
  Range scales (8/32) keep fp8 operands out of the denormal zone and are
  folded into the per-node output scales.
Edge+diag masks are one fused vector op (host bakes the diagonal into the
fp8 mask stream). Row sums are fp8-DR ones-matmuls over slab pairs. Output
tails swap matmul roles (lhsT=aggT chunk, rhs=W.T chunk) so h comes out
node-major: no PE transposes, per-node scales are per-partition scalars.

Phase order hides both column-sum AllReduces under PE work:
  FO_s2d | P1(out) -> AR(out) | P1(in) -> AR(in) | FO_d2s | SO_out | SO_in.
"""
import numpy as np
import ml_dtypes
from contextlib import ExitStack

N = 4096
P = 128
KC = N // P          # 32 k-chunks
JC = KC // 2         # 16 DoubleRow pair-chunks
B = 512              # rows per core
MC = B // P          # 4 row chunks per core
D = 256
DH = D // P          # 2 feature chunks
NCORES = 8
FO_S = 8.0           # fp8 range scale for first-order x
SO_S = 32.0          # fp8 range scale for second-order x

_CACHE = {}


def _build_nc():
    import concourse.bacc as bacc
    import concourse.mybir as mybir
    import concourse.tile as tile
    from concourse.alu_op_type import AluOpType as op
    import bass_rust
    AF = bass_rust.ActivationFunctionType
    AX = bass_rust.AxisListType
    f32 = mybir.dt.float32
    bf16 = mybir.dt.bfloat16
    f8 = mybir.dt.float8e4
    DR = mybir.MatmulPerfMode.DoubleRow

    nc = bacc.Bacc("TRN2", num_devices=NCORES)

    a_strips = nc.dram_tensor("a_strips", [KC, P, KC, P], f8, kind="ExternalInput")
    at_strips = nc.dram_tensor("at_strips", [KC, P, KC, P], f8, kind="ExternalInput")
    acol8_d = nc.dram_tensor("acol8", [N, B], f8, kind="ExternalInput")
    atcol8_d = nc.dram_tensor("atcol8", [N, B], f8, kind="ExternalInput")
    amask8_d = nc.dram_tensor("amask8", [N, B], f8, kind="ExternalInput")
    atmask8_d = nc.dram_tensor("atmask8", [N, B], f8, kind="ExternalInput")
    xbf_d = nc.dram_tensor("xbf", [N, D], bf16, kind="ExternalInput")
    xf = {}
    for t in ("s2d", "d2s"):
        for h in ("hi", "lo"):
            # pair-packed [j, p, r, d]: 512B contiguous per partition per DMA
            xf[(t, h)] = nc.dram_tensor(f"x_{t}_{h}", [JC, P, 2, D], f8,
                                        kind="ExternalInput")
    oa_s2d_d = nc.dram_tensor("oa_s2d", [P, MC], f32, kind="ExternalInput")
    oa_d2s_d = nc.dram_tensor("oa_d2s", [P, MC], f32, kind="ExternalInput")
    wsrcT_d = nc.dram_tensor("wsrcT", [D, D], bf16, kind="ExternalInput")
    wdstT_d = nc.dram_tensor("wdstT", [D, D], bf16, kind="ExternalInput")
    y_d = nc.dram_tensor("y", [B, D], f32, kind="ExternalOutput")

    with tile.TileContext(nc) as tc:
        with ExitStack() as ctx:
            cpool = ctx.enter_context(tc.tile_pool(name="const", bufs=1))
            strips = ctx.enter_context(tc.tile_pool(name="strips", bufs=13))
            xw = ctx.enter_context(tc.tile_pool(name="xw", bufs=8))
            tiny = ctx.enter_context(tc.tile_pool(name="tiny", bufs=1))
            ps_fo = ctx.enter_context(tc.tile_pool(name="ps_fo", bufs=2, space="PSUM"))
            ps_c = ctx.enter_context(tc.tile_pool(name="ps_c", bufs=2, space="PSUM"))
            ps_rs = ctx.enter_context(tc.tile_pool(name="ps_rs", bufs=2, space="PSUM"))
            ps_y = ctx.enter_context(tc.tile_pool(name="ps_y", bufs=2, space="PSUM"))
            dram = ctx.enter_context(tc.tile_pool(name="dram", bufs=1, space="DRAM"))

            # ---- resident constants ----
            acol8_sb = cpool.tile([P, KC, B], f8, name="acol8_sb")
            atcol8_sb = cpool.tile([P, KC, B], f8, name="atcol8_sb")
            for j in range(JC):
                nc.gpsimd.dma_start(
                    out=atcol8_sb[:, 2 * j:2 * j + 2, :],
                    in_=atcol8_d.rearrange("(kc p) j -> p kc j", p=P)[:, 2 * j:2 * j + 2, :])
            for j in range(JC):
                nc.gpsimd.dma_start(
                    out=acol8_sb[:, 2 * j:2 * j + 2, :],
                    in_=acol8_d.rearrange("(kc p) j -> p kc j", p=P)[:, 2 * j:2 * j + 2, :])
            oa_sb = {}
            for name, dd in (("s2d", oa_s2d_d), ("d2s", oa_d2s_d)):
                t = cpool.tile([P, MC], f32, tag=f"oa_{name}", name=f"oa_{name}")
                nc.scalar.dma_start(out=t[:], in_=dd[:])
                oa_sb[name] = t
            w_sb = {}
            for name, dd in (("src", wsrcT_d), ("dst", wdstT_d)):
                t = cpool.tile([P, DH, D], bf16, tag=f"w_{name}", name=f"w_{name}")
                nc.scalar.dma_start(out=t[:], in_=dd.rearrange("(kc p) j -> p kc j", p=P))
                w_sb[name] = t
            ones2 = cpool.tile([P, 2, 16], f8, name="ones2")
            nc.vector.memset(ones2[:], 1.0)

            mc_sb = {"in": cpool.tile([P, KC, B], f8, tag="mcin", name="mcin"),
                     "out": cpool.tile([P, KC, B], f8, tag="mcout", name="mcout")}
            xfs = {(t, h): cpool.tile([P, JC, 2, D], f8, tag=f"xfs_{t}_{h}",
                                      name=f"xfs_{t}_{h}")
                   for t in ("s2d", "d2s") for h in ("hi", "lo")}
            xbf_sb = cpool.tile([P, KC, D], bf16, name="xbf_sb")
            nc.gpsimd.dma_start(out=xbf_sb[:],
                                in_=xbf_d.rearrange("(kc p) d -> p kc d", p=P))
            TERMS = ("fo_s2d", "fo_d2s", "so_in", "so_out")
            aggT = {t: cpool.tile([P, DH, B], bf16, tag=f"agg_{t}", name=f"agg_{t}")
                    for t in TERMS}
            colp = {s: cpool.tile([P, KC], bf16, tag=f"colp_{s}", name=f"colp_{s}")
                    for s in ("in", "out")}
            iso_sb = {s: cpool.tile([P, KC], f32, tag=f"iso_{s}", name=f"iso_{s}")
                      for s in ("in", "out")}
            oso_sb = {s: cpool.tile([P, MC], f32, tag=f"oso_{s}", name=f"oso_{s}")
                      for s in ("in", "out")}
            ysb = cpool.tile([P, MC, D], f32)

            cc = {s: {"i": dram.tile([N], bf16, tag=f"cc_i_{s}", name=f"cc_i_{s}"),
                      "o": dram.tile([N], bf16, tag=f"cc_o_{s}", name=f"cc_o_{s}")}
                  for s in ("in", "out")}
            oso_dram = dram.tile([2, B], f32)

            from concourse.tile_rust import add_dep_helper
            ev_trace = {}

            # ============ phase 1: C blocks + mask + degree sums ============
            def phase1(side, strips_d, col8sb, mask_d):
                """C = (Mt@M)[:, Rc] via fp8 DoubleRow; mask+evict fused (host
                bakes the diagonal into mask_d); rowsums as fp8-DR ones-matmul."""
                mc = mc_sb[side]
                rs = ps_rs.tile([1, B], f32, tag="rs", name=f"rs_{side}")
                for i in range(KC):
                    strip = strips.tile([P, KC, P], f8, tag="strip", name="strip")
                    # alternate queues: one queue can't sustain the strip rate
                    (nc.sync if i % 2 == 0 else nc.scalar).dma_start(
                        out=strip[:], in_=strips_d[i])
                    mchk = strips.tile([P, B], f8, tag="mchk", name="mchk", bufs=8)
                    nc.scalar.dma_start(out=mchk[:], in_=mask_d[i * P:(i + 1) * P, :])
                    cps = ps_c.tile([P, B], f32, tag="c", name="cps")
                    for j in range(JC):
                        nc.tensor.matmul(cps[:], lhsT=strip[:, 2 * j:2 * j + 2, :],
                                         rhs=col8sb[:, 2 * j:2 * j + 2, :],
                                         perf_mode=DR,
                                         start=(j == 0), stop=(j == JC - 1))
                    # fused evict: zero where edge or diagonal (mask != 0)
                    mk = nc.vector.scalar_tensor_tensor(out=mc[:, i, :], in0=mchk[:],
                                                        scalar=0.0, in1=cps[:],
                                                        op0=op.is_equal, op1=op.mult)
                    ev_trace[(side, i)] = mk
                    # partial column sums (free-dim reduce, bf16 to halve AR)
                    with nc.allow_low_precision(reason="colsums feed fp8-quantized iso; bf16 ample"):
                        nc.vector.reduce_sum(colp[side][:, i:i + 1], mc[:, i, :], axis=AX.X)
                    # row-sum ones-matmul over slab pairs, fp8 DoubleRow
                    if i % 2 == 1:
                        nc.tensor.matmul(rs[:], lhsT=ones2[:, :, :1],
                                         rhs=mc[:, i - 1:i + 1, :], perf_mode=DR,
                                         start=(i == 1), stop=(i == KC - 1))
                # o_so = (0.15/SO_S) * rsqrt(rowsum) * (rowsum > 0)
                ind = tiny.tile([1, B], f32, tag=f"rind_{side}", name=f"rind_{side}")
                nc.vector.tensor_scalar(out=ind[:], in0=rs[:], scalar1=0.0,
                                        scalar2=None, op0=op.is_gt)
                val = tiny.tile([1, B], f32, tag=f"rval_{side}", name=f"rval_{side}")
                nc.vector.tensor_scalar(out=val[:], in0=rs[:], scalar1=1e-30,
                                        scalar2=None, op0=op.max)
                nc.scalar.activation(out=val[:], in_=val[:], func=AF.Sqrt,
                                     scale=(SO_S / 0.15) ** 2)
                nc.vector.reciprocal(out=val[:], in_=val[:])
                nc.vector.tensor_tensor(out=val[:], in0=val[:], in1=ind[:], op=op.mult)
                si = 0 if side == "in" else 1
                nc.gpsimd.dma_start(out=oso_dram[si], in_=val[:])
                nc.gpsimd.dma_start(out=oso_sb[side][:],
                                    in_=oso_dram[si].rearrange("(mc p) -> p mc", p=P))
                # ship partial colsums + AllReduce (overlapped with later PE work)
                nc.gpsimd.dma_start(out=cc[side]["i"].rearrange("(kc p) -> p kc", p=P),
                                    in_=colp[side][:])
                with nc.allow_low_precision(reason="colsums feed fp8-quantized iso; bf16 ample"):
                    nc.gpsimd.collective_compute(
                        "AllReduce", mybir.AluOpType.add,
                        replica_groups=[list(range(NCORES))],
                        ins=[cc[side]["i"].opt()], outs=[cc[side]["o"].opt()])

            def iso_prep(side, gate):
                """iso = SO_S * rsqrt(colsum) gated; `gate` keeps the readback
                from stalling the vector FIFO on collective latency."""
                raw = tiny.tile([P, KC], f32, tag=f"israw_{side}", name=f"israw_{side}")
                dma = nc.gpsimd.dma_start(out=raw[:],
                                          in_=cc[side]["o"].rearrange("(kc p) -> p kc", p=P))
                if gate is not None:
                    add_dep_helper(dma.ins, gate.ins, reason="iso readback after vector work")
                ind = tiny.tile([P, KC], f32, tag=f"isind_{side}", name=f"isind_{side}")
                nc.vector.tensor_scalar(out=ind[:], in0=raw[:], scalar1=0.0,
                                        scalar2=None, op0=op.is_gt)
                nc.vector.tensor_scalar(out=raw[:], in0=raw[:], scalar1=1e-30,
                                        scalar2=None, op0=op.max)
                nc.scalar.activation(out=raw[:], in_=raw[:], func=AF.Sqrt,
                                     scale=1.0 / (SO_S * SO_S))
                nc.vector.reciprocal(out=raw[:], in_=raw[:])
                nc.vector.tensor_tensor(out=iso_sb[side][:], in0=raw[:], in1=ind[:],
                                        op=op.mult)

            # SO SpMM: single-fp8 x*iso lhsT against resident fp8 mc, DoubleRow.
            # The two split ops of each pair go to different engines.
            def spmm_so(side, ps):
                scale, rhs_sb = iso_sb[side], mc_sb[side]
                last = None
                for j in range(JC):
                    x8 = xw.tile([P, 2, D], f8, tag="x8", name="x8")
                    for r in range(2):
                        k = 2 * j + r
                        last = nc.vector.tensor_scalar(out=x8[:, r, :],
                                                       in0=xbf_sb[:, k, :],
                                                       scalar1=scale[:, k:k + 1],
                                                       scalar2=None, op0=op.mult)
                    for dh in range(DH):
                        nc.tensor.matmul(ps[dh][:],
                                         lhsT=x8[:, :, dh * P:(dh + 1) * P],
                                         rhs=rhs_sb[:, 2 * j:2 * j + 2, :],
                                         perf_mode=DR,
                                         start=(j == 0), stop=(j == JC - 1))
                return last

            # FO SpMM: host-split hi/lo fp8 lhsT, prefetched resident in SBUF
            # during phase 1 so the matmuls are never DMA-fed
            def spmm_fo(term, rhs_sb, ps):
                for j in range(JC):
                    for dh in range(DH):
                        for half, h in enumerate(("hi", "lo")):
                            nc.tensor.matmul(ps[dh][:],
                                             lhsT=xfs[(term, h)][:, j, :, dh * P:(dh + 1) * P],
                                             rhs=rhs_sb[:, 2 * j:2 * j + 2, :],
                                             perf_mode=DR,
                                             start=(j == 0 and half == 0),
                                             stop=(j == JC - 1 and half == 1))

            # output tail: h[Rc] = aggT.T @ W.T, node-major, no transposes
            TW = {"fo_s2d": "src", "fo_d2s": "dst", "so_out": "src", "so_in": "dst"}

            def term_tail(term, first, emit_y=False):
                w = w_sb[TW[term]]
                ot = {"fo_s2d": oa_sb["s2d"], "fo_d2s": oa_sb["d2s"],
                      "so_out": oso_sb["out"], "so_in": oso_sb["in"]}[term]
                for mh in range(MC):
                    g = ps_y.tile([P, D], f32, tag="y", name="gy")
                    for kh in range(DH):
                        nc.tensor.matmul(g[:], lhsT=aggT[term][:, kh, mh * P:(mh + 1) * P],
                                         rhs=w[:, kh, :],
                                         start=(kh == 0), stop=(kh == DH - 1))
                    dst = ysb[:, mh, :]
                    if first:
                        nc.vector.tensor_scalar(out=dst, in0=g[:],
                                                scalar1=ot[:, mh:mh + 1],
                                                scalar2=None, op0=op.mult)
                    else:
                        nc.vector.scalar_tensor_tensor(out=dst, in0=g[:],
                                                       scalar=ot[:, mh:mh + 1],
                                                       in1=dst, op0=op.mult,
                                                       op1=op.add)
                    if emit_y:
                        nc.gpsimd.dma_start(
                            out=y_d.rearrange("(mc p) d -> p mc d", p=P)[:, mh, :],
                            in_=ysb[:, mh, :])

            # ================= emission order =================
            # Phase 1 first: strips own the early HBM bandwidth; the FO SpMMs
            # run at the end where DMA queues are idle and cover AR(in).
            phase1("out", at_strips, atcol8_sb, atmask8_d)  # ends with AR(out) kickoff
            # FO x-streams prefetch on the idle gpsimd queue, flowing under P1(in)
            for t in ("s2d", "d2s"):
                for h in ("hi", "lo"):
                    for j in range(JC):
                        nc.gpsimd.dma_start(out=xfs[(t, h)][:, j, :, :],
                                            in_=xf[(t, h)][j])
            phase1("in", a_strips, acol8_sb, amask8_d)      # covers AR(out); kicks AR(in)

            iso_prep("out", gate=ev_trace[("in", KC - 8)])
            soout_ps = [ps_fo.tile([P, B], f32, tag="fo", name=f"soout_{dh}")
                        for dh in range(DH)]
            ev2 = spmm_so("out", soout_ps)
            for dh in range(DH):
                nc.vector.tensor_copy(out=aggT["so_out"][:, dh, :], in_=soout_ps[dh][:])
            term_tail("so_out", first=True)

            # iso(in) early: its vector chain runs under the FO SpMMs
            iso_prep("in", gate=ev2)

            fo_ps = {(t, dh): ps_fo.tile([P, B], f32, tag="fo", name=f"fo_{t}_{dh}")
                     for t in ("s2d", "d2s") for dh in range(DH)}
            spmm_fo("s2d", atcol8_sb, [fo_ps[("s2d", dh)] for dh in range(DH)])
            for dh in range(DH):
                nc.vector.tensor_copy(out=aggT["fo_s2d"][:, dh, :],
                                      in_=fo_ps[("s2d", dh)][:])
            term_tail("fo_s2d", first=False)

            spmm_fo("d2s", acol8_sb, [fo_ps[("d2s", dh)] for dh in range(DH)])
            for dh in range(DH):
                nc.vector.tensor_copy(out=aggT["fo_d2s"][:, dh, :],
                                      in_=fo_ps[("d2s", dh)][:])
            term_tail("fo_d2s", first=False)

            soin_ps = [ps_fo.tile([P, B], f32, tag="fo", name=f"soin_{dh}")
                       for dh in range(DH)]
            spmm_so("in", soin_ps)
            for dh in range(DH):
                nc.vector.tensor_copy(out=aggT["so_in"][:, dh, :], in_=soin_ps[dh][:])
            term_tail("so_in", first=False, emit_y=True)

    nc.finalize()
    return nc


def _host_prep(x, edge_index):
    ei = np.asarray(edge_index).astype(np.int64)
    lin = ei[0] * N + ei[1]
    uniq = np.unique(lin)
    A = np.zeros(N * N, np.float32)
    A[uniq] = 1.0
    A = A.reshape(N, N)
    dr = np.bincount((uniq // N).astype(np.int64), minlength=N).astype(np.float64)
    dc = np.bincount((uniq % N).astype(np.int64), minlength=N).astype(np.float64)

    def rnorm(d):
        return np.where(d > 0, 1.0 / np.sqrt(np.maximum(d, 1e-30)), 0.0).astype(np.float32)

    rdr, rdc = rnorm(dr), rnorm(dc)
    f8 = ml_dtypes.float8_e4m3
    A8 = A.astype(f8)
    At8 = np.ascontiguousarray(A8.T)
    # masks with the diagonal baked in (nonzero => zero the C entry)
    Am = A.copy()
    np.fill_diagonal(Am, 1.0)
    Am8 = Am.astype(f8)
    Atm8 = np.ascontiguousarray(Am8.T)
    a_strips = np.ascontiguousarray(A8.reshape(KC, P, KC, P).transpose(2, 1, 0, 3))
    at_strips = np.ascontiguousarray(At8.reshape(KC, P, KC, P).transpose(2, 1, 0, 3))
    mats = {"A8": A8, "At8": At8, "Am8": Am8, "Atm8": Atm8}
    return mats, a_strips, at_strips, rdr, rdc


def _fo_split(x, scale):
    f8 = ml_dtypes.float8_e4m3
    xs = (x * (FO_S * scale)[:, None]).astype(np.float32)
    hi = xs.astype(f8)
    lo = (xs - hi.astype(np.float32)).astype(f8)

    def pack(a):
        # [N, D] -> [JC, P, 2, D]: DMA-friendly pair-packed layout
        return np.ascontiguousarray(
            a.reshape(JC, 2, P, D).transpose(0, 2, 1, 3))

    return pack(hi), pack(lo)


def _in_maps(x, mats, a_strips, at_strips, rdr, rdc, wsrcT, wdstT):
    bf16 = ml_dtypes.bfloat16
    xs2d_hi, xs2d_lo = _fo_split(x, rdc)
    xd2s_hi, xd2s_lo = _fo_split(x, rdr)
    xbf = x.astype(bf16)
    w_src = np.ascontiguousarray(wsrcT).astype(bf16)
    w_dst = np.ascontiguousarray(wdstT).astype(bf16)
    maps = []
    for c in range(NCORES):
        sl = slice(c * B, (c + 1) * B)
        maps.append({
            "a_strips": a_strips, "at_strips": at_strips,
            "acol8": np.ascontiguousarray(mats["A8"][:, sl]),
            "atcol8": np.ascontiguousarray(mats["At8"][:, sl]),
            "amask8": np.ascontiguousarray(mats["Am8"][:, sl]),
            "atmask8": np.ascontiguousarray(mats["Atm8"][:, sl]),
            "xbf": xbf,
            "x_s2d_hi": xs2d_hi, "x_s2d_lo": xs2d_lo,
            "x_d2s_hi": xd2s_hi, "x_d2s_lo": xd2s_lo,
            "oa_s2d": np.ascontiguousarray((0.35 / FO_S * rdr[sl]).reshape(MC, P).T),
            "oa_d2s": np.ascontiguousarray((0.35 / FO_S * rdc[sl]).reshape(MC, P).T),
            "wsrcT": w_src, "wdstT": w_dst,
        })
    return maps


def kernel(x, edge_index, W_src, b_src, W_dst, b_dst):
    from concourse.bass_utils import run_bass_kernel_spmd

    x = np.asarray(x, dtype=np.float32)
    W_src = np.asarray(W_src, dtype=np.float32)
    W_dst = np.asarray(W_dst, dtype=np.float32)
    b_src = np.asarray(b_src, dtype=np.float32)
    b_dst = np.asarray(b_dst, dtype=np.float32)

    mats, a_strips, at_strips, rdr, rdc = _host_prep(x, edge_index)
    in_maps = _in_maps(x, mats, a_strips, at_strips, rdr, rdc,
                       np.ascontiguousarray(W_src.T), np.ascontiguousarray(W_dst.T))

    if "nc" not in _CACHE:
        _CACHE["nc"] = _build_nc()
    res = run_bass_kernel_spmd(_CACHE["nc"], in_maps, list(range(NCORES)))
    y = np.concatenate([res.results[c]["y"] for c in range(NCORES)], axis=0)
    y = y + 0.5 * (b_src + b_dst)[None, :]
    return np.ascontiguousarray(y.astype(np.float32))
